# revision 10
# baseline (speedup 1.0000x reference)
"""Greedy bipartite matching (NMS-style) Bass kernel for TRN2 — v2.

Algorithm: iterated locally-dominant (mutual-max) matching == sequential
greedy matching.  v2 replaces the index-key mutual test with a CELL-based
test fused into two custom DVE ops:

  MASKED_MAX : W = W * aliveB (in place), accum_out = per-partition max
  MUTUAL     : count = sum_j [ W[p,j] == ownmax_p  &  W[p,j] == othermaxB[j] ]

A cell is matched iff it equals both its row max and its column max
(checked per cell, so cross-row/col duplicate values cannot alias).  The
matched VALUE per row is recorded (deathR); the output permutation row is
recovered at the end as one_hot = (s_orig == deathR), which is exact
unless the death value is duplicated inside its own row (rare; caught by
the host structural check and recomputed exactly).

Stages per matrix: 2 full rounds (512-wide, round 1 uses the cheap
tensor_scalar max-accum since nothing is masked yet), compact to the
2-block L1 layout (dense ids, 160-wide windows), 1 L1 round, compact to a
single [128,128] tile, 5 tail rounds, host completes the ~4-row residual
exactly (greedy prefix property).  No column-id bookkeeping is needed
anywhere; row-id maps (rid1p / rid) translate compact-space death values
back to original rows via one-hot matmuls at output time.

Emission is interleaved over groups of G matrices so engines overlap.
"""

import numpy as np
import concourse.bass as bass
import concourse.bacc as bacc
import concourse.mybir as mybir
from concourse.tile import TileContext
from concourse import library_config

FP = mybir.dt.float32
AL = mybir.AluOpType

# ---------------------------------------------------------------------------
# custom DVE ops (registered into concourse.dve_ops at import; this is the
# documented extension path for ant custom ops)
# ---------------------------------------------------------------------------
import concourse.dve_ops as _dvo
from concourse.dve_ops import DveOp as _DveOp
from concourse.dve_spec import (Spec as _Spec, Src0 as _S0, Src1 as _S1,
                                C0 as _C0, maxx as _maxx, eq as _eq,
                                lower as _lower, AluOp as _AluOp)
from concourse.dve_uop import DveOpSpec as _DveOpSpec


def _register_op(name, spec, subdim=False):
    if name in _dvo._SUB_OPCODE_FOR_NAME:
        for op in _dvo.OPS:
            if op.name == name:
                return op
    shas = {}
    for ver in ("v3",):
        uops = _lower(spec, ver=ver)
        shas[ver] = _DveOpSpec(name=name, opcode=1, uops=uops,
                               rd1_en=True).sha(ver)
    op = _DveOp(name, spec, subdim, shas)
    _dvo.OPS.append(op)
    _dvo._SUB_OPCODE_FOR_NAME[name] = (_dvo._CUSTOM_DVE_ROW_BASE
                                       + len(_dvo.OPS) - 1)
    _dvo.CUSTOM_DVE_SPECS[name] = spec
    return op


def _ref_masked_max(in0, in1, c0, c1, c2):
    b = (in0 * in1).astype(np.float32)
    return b, b.reshape(b.shape[0], -1).max(axis=-1, keepdims=True)


def _ref_mutual(in0, in1, c0, c1, c2):
    b = ((in0 == c0) & (in0 == in1)).astype(np.float32)
    return b, b.reshape(b.shape[0], -1).sum(axis=-1, keepdims=True)


MASKED_MAX = _register_op(
    "NMS_MASKED_MAX",
    _Spec(body=_S0 * _S1, accum=_maxx, reference=_ref_masked_max))

MUTUAL = _register_op(
    "NMS_MUTUAL",
    _Spec(body=_eq(_S0, _C0) & _eq(_S0, _S1), accum=_AluOp.ADD,
          reference=_ref_mutual))


# ---- const layout (free-dim offsets into the [128, CONST_W] consts tensor)
OFF_I128 = 0         # [128,128] identity
OFF_UT128 = 128      # [128,128] upper-tri (q<=p)
OFF_IOTAF128 = 256   # [128,128] value f
OFF_IOTAF160 = 384   # [128,160] value f
OFF_ONES = 544       # [128,128] ones
OFF_COLID = 672      # [128,4] 128k+p
CONST_W = 676


def make_consts() -> np.ndarray:
    c = np.zeros((128, CONST_W), dtype=np.float32)
    q = np.arange(128)
    c[:, OFF_I128:OFF_I128 + 128] = np.eye(128, dtype=np.float32)
    c[:, OFF_UT128:OFF_UT128 + 128] = (q[:, None] <= q[None, :]).astype(np.float32)
    c[:, OFF_IOTAF128:OFF_IOTAF128 + 128] = q[None, :]
    c[:, OFF_IOTAF160:OFF_IOTAF160 + 160] = np.arange(160)[None, :]
    c[:, OFF_ONES:OFF_ONES + 128] = 1.0
    for k in range(4):
        c[:, OFF_COLID + k] = 128 * k + q
    return c


def build_nms_kernel(nc: bass.Bass, out_ap, s_ap, consts_ap, n_mat: int,
                     tail_rounds: int = 5, group: int = 6):
    with TileContext(nc) as tc:
        with (
            tc.tile_pool(name="consts", bufs=1) as pool_c,
            tc.tile_pool(name="big", bufs=1) as pool_big,
            tc.tile_pool(name="sm", bufs=1) as pool_sm,
            tc.tile_pool(name="vec", bufs=1) as pool_vec,
            tc.tile_pool(name="outp", bufs=1) as pool_out,
            tc.tile_pool(name="ps", bufs=3, space="PSUM") as pool_ps,
            tc.tile_pool(name="psM", bufs=1, space="PSUM") as pool_psM,
            tc.tile_pool(name="psT", bufs=2, space="PSUM") as pool_psT,
            tc.tile_pool(name="psC", bufs=2, space="PSUM") as pool_psC,
        ):
            C = pool_c.tile([128, CONST_W], FP, name="consts", tag="consts")
            nc.sync.dma_start(out=C[:, :], in_=consts_ap[:, :])
            I128 = C[:, OFF_I128:OFF_I128 + 128]
            UT128 = C[:, OFF_UT128:OFF_UT128 + 128]
            iotaF128 = C[:, OFF_IOTAF128:OFF_IOTAF128 + 128]
            iotaF160 = C[:, OFF_IOTAF160:OFF_IOTAF160 + 160]
            ONES = C[:, OFF_ONES:OFF_ONES + 128]
            iotaColId = C[:, OFF_COLID:OFF_COLID + 4]

            nc.gpsimd.load_library(library_config.proxy)
            warm = pool_psT.tile([128, 128], FP, name="warm", tag="pst")
            nc.tensor.transpose(warm[:, :], I128, I128)

            def big(nm, s, w=2048):
                return pool_big.tile([128, w], FP, name=f"{nm}{s}",
                                     tag=f"{nm}{s}")

            def sm(nm, s, w=128):
                return pool_sm.tile([128, w], FP, name=f"{nm}{s}",
                                    tag=f"{nm}{s}")

            def vec(nm, s, w=4, p=128):
                return pool_vec.tile([p, w], FP, name=f"{nm}{s}",
                                     tag=f"{nm}{s}")

            trash = pool_big.tile([128, 512], FP, name="trashG", tag="trashG")
            rowstage = [pool_vec.tile([1, 1024], FP, name=f"rstg{i}",
                                      tag=f"rstg{i}") for i in range(group)]

            def make_state(s):
                st = {}
                st["Wb"] = big("W_", s)          # 4 blocks of [128,512]
                st["Wtb"] = big("Wt_", s)
                st["W"] = [st["Wb"][:, 512 * k:512 * (k + 1)] for k in range(4)]
                st["Wt"] = [st["Wtb"][:, 512 * k:512 * (k + 1)] for k in range(4)]
                st["maxB"] = big("mxB_", s, w=1024)
                st["aliveB"] = big("alB_", s, w=1024)
                st["maxRow"] = rowstage[s]
                st["alvRow"] = rowstage[s]
                for nm in ("rowalive", "colalive", "rowmax", "colmax",
                           "rowmaxG", "colmaxG", "cntR", "cntC", "deathR",
                           "t1", "t2", "t3"):
                    st[nm] = vec(nm + "_", s)
                # compact1 / L1 state
                st["GrT"] = [sm(f"GrT{k}_", s) for k in range(4)]
                st["GcT"] = [sm(f"GcT{k}_", s) for k in range(4)]
                st["rid1p"] = vec("r1p_", s, 2)
                st["rowmax1"] = vec("rm1_", s, 2)
                st["colmax1"] = vec("cm1_", s, 2)
                st["ral1"] = vec("ra1_", s, 2)
                st["cal1"] = vec("ca1_", s, 2)
                st["cnt1R"] = vec("c1R_", s, 2)
                st["cnt1C"] = vec("c1C_", s, 2)
                st["dR1"] = vec("dR1_", s, 2)
                st["u1"] = vec("u1_", s, 2)
                st["u2"] = vec("u2_", s, 2)
                st["scanrow"] = vec("scn_", s, 12, p=1)
                st["scanrow2"] = vec("sc2_", s, 12, p=1)
                # compact2 / tail state (overlay compact1 scratch: GrT/GcT
                # [2] and [3] are dead after compact1's gathers)
                st["Wc"] = st["GrT"][2]
                st["WtC"] = st["GrT"][3]
                st["A2"] = [st["GcT"][2], st["GcT"][3]]
                st["rid"] = vec("rid_", s, 1)
                st["rmC"] = vec("rmC_", s, 1)
                st["cmC"] = vec("cmC_", s, 1)
                st["ralC"] = vec("ralC_", s, 1)
                st["calC"] = vec("calC_", s, 1)
                st["cntRC"] = vec("cRC_", s, 1)
                st["cntCC"] = vec("cCC_", s, 1)
                st["dRC"] = vec("dRC_", s, 1)
                st["v1"] = vec("v1_", s, 1)
                st["v2"] = vec("v2_", s, 1)
                return st

            states = [make_state(s) for s in range(group)]

            def bcast2(vecs_a, vecs_b, rowt, B, wa=512, wb=512):
                """vecs_a -> rowt[0:wa] -> B[:, 0:wa]; vecs_b -> [wa:wa+wb].
                vecs_* is a list of ([128,1] AP, out_off, width<=128)."""
                pra = pool_ps.tile([1, 512], FP, name="ps", tag="ps")
                for v, off, w in vecs_a:
                    nc.tensor.matmul(pra[0:1, off:off + w], v, I128[:, 0:w],
                                     start=True, stop=True)
                nc.scalar.copy(rowt[0:1, 0:wa], pra[0:1, 0:wa])
                prb = pool_ps.tile([1, 512], FP, name="ps", tag="ps")
                for v, off, w in vecs_b:
                    nc.tensor.matmul(prb[0:1, off:off + w], v, I128[:, 0:w],
                                     start=True, stop=True)
                nc.scalar.copy(rowt[0:1, wa:wa + wb], prb[0:1, 0:wb])
                nc.gpsimd.partition_broadcast(B[:, 0:wa + wb],
                                              rowt[0:1, 0:wa + wb])

            # ================= stages =================
            def load(st, m):
                for k in range(4):
                    nc.sync.dma_start(out=st["W"][k][:, :],
                                      in_=s_ap[m, 128 * k:128 * (k + 1), :])
                for r in range(4):
                    pt = pool_psT.tile([128, 512], FP, name="pst", tag="pst")
                    for k in range(4):
                        nc.tensor.transpose(
                            pt[:, 128 * k:128 * (k + 1)],
                            st["W"][k][:, 128 * r:128 * (r + 1)], I128)
                    nc.scalar.copy(st["Wt"][r][:, :], pt[:, :])

            def full_a(st, r):
                W, Wt = st["W"], st["Wt"]
                rowmax, colmax = st["rowmax"], st["colmax"]
                if r == 0:
                    for k in range(4):
                        nc.vector.tensor_scalar(
                            out=trash[:, :], in0=Wt[k][:, :], scalar1=1.0,
                            scalar2=0.0, op0=AL.mult, op1=AL.max,
                            accum_out=colmax[:, k:k + 1])
                    for k in range(4):
                        nc.vector.tensor_scalar(
                            out=trash[:, :], in0=W[k][:, :], scalar1=1.0,
                            scalar2=0.0, op0=AL.mult, op1=AL.max,
                            accum_out=rowmax[:, k:k + 1])
                else:
                    for k in range(4):
                        nc.gpsimd.tensor_tensor(out=Wt[k][:, :],
                                                in0=Wt[k][:, :],
                                                in1=st["aliveB"][:, 512:1024],
                                                op=AL.mult)
                        nc.vector.tensor_scalar(
                            out=trash[:, :], in0=Wt[k][:, :], scalar1=1.0,
                            scalar2=0.0, op0=AL.mult, op1=AL.max,
                            accum_out=colmax[:, k:k + 1])
                    for k in range(4):
                        nc.gpsimd.tensor_tensor(out=W[k][:, :],
                                                in0=W[k][:, :],
                                                in1=st["aliveB"][:, 0:512],
                                                op=AL.mult)
                        nc.vector.tensor_scalar(
                            out=trash[:, :], in0=W[k][:, :], scalar1=1.0,
                            scalar2=0.0, op0=AL.mult, op1=AL.max,
                            accum_out=rowmax[:, k:k + 1])
                    nc.vector.tensor_tensor(out=st["colmaxG"][:, :],
                                            in0=colmax[:, :],
                                            in1=st["colalive"][:, :],
                                            op=AL.mult)
                    nc.vector.tensor_tensor(out=st["rowmaxG"][:, :],
                                            in0=rowmax[:, :],
                                            in1=st["rowalive"][:, :],
                                            op=AL.mult)

            def full_b(st, r):
                rmG = st["rowmax"] if r == 0 else st["rowmaxG"]
                cmG = st["colmax"] if r == 0 else st["colmaxG"]
                bcast2([(cmG[:, k:k + 1], 128 * k, 128) for k in range(4)],
                       [(rmG[:, k:k + 1], 128 * k, 128) for k in range(4)],
                       st["maxRow"], st["maxB"])

            def full_c(st, r):
                W, Wt = st["W"], st["Wt"]
                rmG = st["rowmax"] if r == 0 else st["rowmaxG"]
                cmG = st["colmax"] if r == 0 else st["colmaxG"]
                for k in range(4):
                    nc.vector._custom_dve(
                        MUTUAL, out=trash[:, :], in0=W[k][:, :],
                        in1=st["maxB"][:, 0:512], s0=rmG[:, k:k + 1],
                        accum_out=st["cntR"][:, k:k + 1])
                for k in range(4):
                    nc.vector._custom_dve(
                        MUTUAL, out=trash[:, :], in0=Wt[k][:, :],
                        in1=st["maxB"][:, 512:1024], s0=cmG[:, k:k + 1],
                        accum_out=st["cntC"][:, k:k + 1])
                t1, t2, t3 = st["t1"], st["t2"], st["t3"]
                nc.vector.tensor_scalar(out=t1[:, :], in0=st["cntR"][:, :],
                                        scalar1=0.5, scalar2=None,
                                        op0=AL.is_ge)
                nc.vector.tensor_scalar(out=t2[:, :], in0=st["cntC"][:, :],
                                        scalar1=0.5, scalar2=None,
                                        op0=AL.is_ge)
                if r == 0:
                    nc.vector.tensor_tensor(out=st["deathR"][:, :],
                                            in0=st["rowmax"][:, :],
                                            in1=t1[:, :], op=AL.mult)
                    nc.vector.tensor_scalar(out=st["rowalive"][:, :],
                                            in0=t1[:, :], scalar1=-1.0,
                                            scalar2=1.0, op0=AL.mult,
                                            op1=AL.add)
                    nc.vector.tensor_scalar(out=st["colalive"][:, :],
                                            in0=t2[:, :], scalar1=-1.0,
                                            scalar2=1.0, op0=AL.mult,
                                            op1=AL.add)
                else:
                    nc.vector.tensor_tensor(out=t3[:, :], in0=st["rowmaxG"][:, :],
                                            in1=t1[:, :], op=AL.mult)
                    nc.vector.tensor_tensor(out=st["deathR"][:, :],
                                            in0=st["deathR"][:, :],
                                            in1=t3[:, :], op=AL.add)
                    nc.vector.tensor_scalar(out=t1[:, :], in0=t1[:, :],
                                            scalar1=-1.0, scalar2=1.0,
                                            op0=AL.mult, op1=AL.add)
                    nc.vector.tensor_tensor(out=st["rowalive"][:, :],
                                            in0=st["rowalive"][:, :],
                                            in1=t1[:, :], op=AL.mult)
                    nc.vector.tensor_scalar(out=t2[:, :], in0=t2[:, :],
                                            scalar1=-1.0, scalar2=1.0,
                                            op0=AL.mult, op1=AL.add)
                    nc.vector.tensor_tensor(out=st["colalive"][:, :],
                                            in0=st["colalive"][:, :],
                                            in1=t2[:, :], op=AL.mult)

            def full_d(st, r):
                if r == 0:
                    bcast2([(st["colalive"][:, k:k + 1], 128 * k, 128)
                            for k in range(4)],
                           [(st["rowalive"][:, k:k + 1], 128 * k, 128)
                            for k in range(4)],
                           st["alvRow"], st["aliveB"])

            def block_offsets(alive4, tot, w=4):
                ptot = pool_psM.tile([1, w], FP, name="ps", tag="ps")
                nc.tensor.matmul(ptot[0:1, :], ONES[:, 0:1], alive4[:, :],
                                 start=True, stop=True)
                nc.vector.tensor_copy(tot[0:1, 0:w], ptot[0:1, :])
                nc.vector.tensor_tensor_scan(
                    out=tot[0:1, 4:4 + w], data0=tot[0:1, 0:w],
                    data1=tot[0:1, 0:w],
                    initial=0.0, op0=AL.add, op1=AL.bypass)
                nc.vector.tensor_tensor(out=tot[0:1, 8:8 + w],
                                        in0=tot[0:1, 4:4 + w],
                                        in1=tot[0:1, 0:w], op=AL.subtract)
                pb = pool_psM.tile([128, w], FP, name="ps", tag="ps")
                nc.tensor.matmul(pb[:, :], ONES[0:1, 0:128],
                                 tot[0:1, 8:8 + w], start=True, stop=True)
                return pb

            # ---------- compact 512-space -> L1 2-block space ----------
            def compact1(st):
                ppre = pool_psM.tile([128, 4], FP, name="ps", tag="ps")
                nc.tensor.matmul(ppre[:, :], UT128, st["rowalive"][:, :],
                                 start=True, stop=True)
                posR = st["t1"]
                nc.scalar.copy(posR[:, :], ppre[:, :])
                ppre2 = pool_psM.tile([128, 4], FP, name="ps", tag="ps")
                nc.tensor.matmul(ppre2[:, :], UT128, st["colalive"][:, :],
                                 start=True, stop=True)
                posC = st["t3"]
                nc.scalar.copy(posC[:, :], ppre2[:, :])
                offRB = block_offsets(st["rowalive"], st["scanrow"])
                offCB = block_offsets(st["colalive"], st["scanrow2"])
                nc.vector.tensor_tensor(out=posR[:, :], in0=posR[:, :],
                                        in1=offRB[:, :], op=AL.add)
                nc.vector.tensor_scalar(out=posR[:, :], in0=posR[:, :],
                                        scalar1=-1.0, scalar2=None,
                                        op0=AL.add)
                nc.vector.tensor_tensor(out=posC[:, :], in0=posC[:, :],
                                        in1=offCB[:, :], op=AL.add)
                nc.vector.tensor_scalar(out=posC[:, :], in0=posC[:, :],
                                        scalar1=-1.0, scalar2=None,
                                        op0=AL.add)
                posRm = st["t2"]
                nc.vector.tensor_scalar(out=posRm[:, :], in0=posR[:, :],
                                        scalar1=-128.0, scalar2=None,
                                        op0=AL.add)
                W1, Wt1 = st["W"][0], st["Wt"][0]
                GcTf = [st["W"][2][:, 0:160], st["W"][2][:, 160:320],
                        st["W"][2][:, 320:480], st["W"][3][:, 0:160]]
                Asb = [st["W"][1][:, 0:160], st["W"][1][:, 160:320],
                       st["W"][1][:, 320:480], st["W"][3][:, 160:320]]
                for cb in range(4):
                    nc.vector.tensor_scalar(out=GcTf[cb], in0=iotaF160,
                                            scalar1=posC[:, cb:cb + 1],
                                            scalar2=st["colalive"][:, cb:cb + 1],
                                            op0=AL.is_equal, op1=AL.mult)
                for k in range(4):
                    nc.vector.tensor_scalar(out=st["GrT"][k][:, :],
                                            in0=iotaF128,
                                            scalar1=posR[:, k:k + 1],
                                            scalar2=st["rowalive"][:, k:k + 1],
                                            op0=AL.is_equal, op1=AL.mult)
                    nc.vector.tensor_scalar(out=st["GcT"][k][:, :],
                                            in0=iotaF128,
                                            scalar1=posRm[:, k:k + 1],
                                            scalar2=st["rowalive"][:, k:k + 1],
                                            op0=AL.is_equal, op1=AL.mult)
                for k in range(4):
                    pA = pool_psC.tile([128, 160], FP, name="psA", tag="psA")
                    for cb in range(4):
                        nc.tensor.matmul(pA[:, :],
                                         st["Wt"][cb][:, 128 * k:128 * (k + 1)],
                                         GcTf[cb], start=(cb == 0),
                                         stop=(cb == 3))
                    nc.scalar.copy(Asb[k], pA[:, :])
                for b in range(2):
                    pB = pool_psC.tile([128, 160], FP, name="psB", tag="psA")
                    for k in range(4):
                        G = st["GrT"][k] if b == 0 else st["GcT"][k]
                        nc.tensor.matmul(pB[:, :], G[:, :], Asb[k],
                                         start=(k == 0), stop=(k == 3))
                    nc.scalar.copy(W1[:, 256 * b:256 * b + 160], pB[:, :])
                # zero the stale pad regions so the Wt1 transposes below see
                # zeros (phantom slots then self-kill through the 0==0
                # channel at the L1 round and never reach compact2)
                nc.vector.memset(W1[:, 160:256], 0.0)
                nc.vector.memset(W1[:, 416:512], 0.0)
                ptW = pool_psT.tile([128, 512], FP, name="pst", tag="pst")
                for bp in range(2):
                    for seg in range(2):
                        nc.tensor.transpose(
                            ptW[:, 256 * bp + 128 * seg:256 * bp + 128 * seg + 128],
                            W1[:, 256 * seg + 128 * bp:256 * seg + 128 * bp + 128],
                            I128)
                nc.scalar.copy(Wt1[:, :], ptW[:, :])
                for b in range(2):
                    pr_ = pool_psM.tile([128, 1], FP, name="ps", tag="ps")
                    for k in range(4):
                        G = st["GrT"][k] if b == 0 else st["GcT"][k]
                        nc.tensor.matmul(pr_[:, :], G[:, :],
                                         iotaColId[:, k:k + 1],
                                         start=(k == 0), stop=(k == 3))
                    nc.scalar.copy(st["rid1p"][:, b:b + 1], pr_[:, :])
                nc.vector.memset(st["dR1"][:, :], 0.0)

            # ---------- the single L1 round (everything alive) ----------
            def l1_a(st):
                W1, Wt1 = st["W"][0], st["Wt"][0]
                rm1, cm1 = st["rowmax1"], st["colmax1"]
                for b in range(2):
                    nc.vector.tensor_scalar(
                        out=trash[:, 0:160], in0=Wt1[:, 256 * b:256 * b + 160],
                        scalar1=1.0, scalar2=0.0, op0=AL.mult, op1=AL.max,
                        accum_out=cm1[:, b:b + 1])
                for b in range(2):
                    nc.vector.tensor_scalar(
                        out=trash[:, 0:160], in0=W1[:, 256 * b:256 * b + 160],
                        scalar1=1.0, scalar2=0.0, op0=AL.mult, op1=AL.max,
                        accum_out=rm1[:, b:b + 1])

            def l1_b(st):
                rm1, cm1 = st["rowmax1"], st["colmax1"]
                bcast2([(cm1[:, 0:1], 0, 128), (cm1[:, 1:2], 128, 32)],
                       [(rm1[:, 0:1], 0, 128), (rm1[:, 1:2], 128, 32)],
                       st["maxRow"], st["maxB"], wa=160, wb=160)

            def l1_c(st):
                W1, Wt1 = st["W"][0], st["Wt"][0]
                rm1, cm1 = st["rowmax1"], st["colmax1"]
                for b in range(2):
                    nc.vector._custom_dve(
                        MUTUAL, out=trash[:, 0:160],
                        in0=W1[:, 256 * b:256 * b + 160],
                        in1=st["maxB"][:, 0:160], s0=rm1[:, b:b + 1],
                        accum_out=st["cnt1R"][:, b:b + 1])
                for b in range(2):
                    nc.vector._custom_dve(
                        MUTUAL, out=trash[:, 0:160],
                        in0=Wt1[:, 256 * b:256 * b + 160],
                        in1=st["maxB"][:, 160:320], s0=cm1[:, b:b + 1],
                        accum_out=st["cnt1C"][:, b:b + 1])
                u1, u2 = st["u1"], st["u2"]
                nc.vector.tensor_scalar(out=u1[:, :], in0=st["cnt1R"][:, :],
                                        scalar1=0.5, scalar2=None,
                                        op0=AL.is_ge)
                nc.vector.tensor_scalar(out=u2[:, :], in0=st["cnt1C"][:, :],
                                        scalar1=0.5, scalar2=None,
                                        op0=AL.is_ge)
                nc.vector.tensor_tensor(out=st["dR1"][:, :], in0=rm1[:, :],
                                        in1=u1[:, :], op=AL.mult)
                nc.vector.tensor_scalar(out=st["ral1"][:, :], in0=u1[:, :],
                                        scalar1=-1.0, scalar2=1.0,
                                        op0=AL.mult, op1=AL.add)
                nc.vector.tensor_scalar(out=st["cal1"][:, :], in0=u2[:, :],
                                        scalar1=-1.0, scalar2=1.0,
                                        op0=AL.mult, op1=AL.add)

            # ---------- compact L1 -> tail [128,128] space ----------
            def compact2(st):
                W1, Wt1 = st["W"][0], st["Wt"][0]
                pp1 = pool_psM.tile([128, 2], FP, name="ps", tag="ps")
                nc.tensor.matmul(pp1[:, :], UT128, st["ral1"][:, :],
                                 start=True, stop=True)
                posR = st["u1"]
                nc.scalar.copy(posR[:, 0:2], pp1[:, :])
                pp2 = pool_psM.tile([128, 2], FP, name="ps", tag="ps")
                nc.tensor.matmul(pp2[:, :], UT128, st["cal1"][:, :],
                                 start=True, stop=True)
                posC = st["u2"]
                nc.scalar.copy(posC[:, 0:2], pp2[:, :])
                offRB = block_offsets(st["ral1"], st["scanrow"], w=2)
                offCB = block_offsets(st["cal1"], st["scanrow2"], w=2)
                nc.vector.tensor_tensor(out=posR[:, 0:2], in0=posR[:, 0:2],
                                        in1=offRB[:, :], op=AL.add)
                nc.vector.tensor_scalar(out=posR[:, 0:2], in0=posR[:, 0:2],
                                        scalar1=-1.0, scalar2=None,
                                        op0=AL.add)
                nc.vector.tensor_tensor(out=posC[:, 0:2], in0=posC[:, 0:2],
                                        in1=offCB[:, :], op=AL.add)
                nc.vector.tensor_scalar(out=posC[:, 0:2], in0=posC[:, 0:2],
                                        scalar1=-1.0, scalar2=None,
                                        op0=AL.add)
                for b in range(2):
                    nc.vector.tensor_scalar(out=st["GrT"][b][:, :],
                                            in0=iotaF128,
                                            scalar1=posR[:, b:b + 1],
                                            scalar2=st["ral1"][:, b:b + 1],
                                            op0=AL.is_equal, op1=AL.mult)
                    nc.vector.tensor_scalar(out=st["GcT"][b][:, :],
                                            in0=iotaF128,
                                            scalar1=posC[:, b:b + 1],
                                            scalar2=st["cal1"][:, b:b + 1],
                                            op0=AL.is_equal, op1=AL.mult)
                for b in range(2):
                    pA = pool_psT.tile([128, 128], FP, name="pst", tag="pst")
                    for cb in range(2):
                        nc.tensor.matmul(
                            pA[:, :],
                            Wt1[:, 256 * cb + 128 * b:256 * cb + 128 * b + 128],
                            st["GcT"][cb][:, :],
                            start=(cb == 0), stop=(cb == 1))
                    nc.scalar.copy(st["A2"][b][:, :], pA[:, :])
                pW = pool_psM.tile([128, 128], FP, name="ps", tag="ps")
                for b in range(2):
                    nc.tensor.matmul(pW[:, :], st["GrT"][b][:, :],
                                     st["A2"][b][:, :],
                                     start=(b == 0), stop=(b == 1))
                nc.scalar.copy(st["Wc"][:, :], pW[:, :])
                ptc = pool_psM.tile([128, 128], FP, name="ps", tag="ps")
                nc.tensor.transpose(ptc[:, :], st["Wc"][:, :], I128)
                nc.scalar.copy(st["WtC"][:, :], ptc[:, :])
                prid = pool_psM.tile([128, 1], FP, name="ps", tag="ps")
                for b in range(2):
                    nc.tensor.matmul(prid[:, :], st["GrT"][b][:, :],
                                     st["rid1p"][:, b:b + 1],
                                     start=(b == 0), stop=(b == 1))
                nc.scalar.copy(st["rid"][:, :], prid[:, :])
                nc.vector.memset(st["dRC"][:, :], 0.0)
                nc.vector.memset(st["ralC"][:, :], 1.0)
                nc.vector.memset(st["calC"][:, :], 1.0)

            def tail_a(st, r):
                Wc, WtC = st["Wc"], st["WtC"]
                rmC, cmC = st["rmC"], st["cmC"]
                if r == 0:
                    nc.vector.tensor_scalar(
                        out=trash[:, 0:128], in0=WtC[:, :], scalar1=1.0,
                        scalar2=0.0, op0=AL.mult, op1=AL.max,
                        accum_out=cmC[:, 0:1])
                    nc.vector.tensor_scalar(
                        out=trash[:, 0:128], in0=Wc[:, :], scalar1=1.0,
                        scalar2=0.0, op0=AL.mult, op1=AL.max,
                        accum_out=rmC[:, 0:1])
                else:
                    nc.vector._custom_dve(
                        MASKED_MAX, out=WtC[:, :], in0=WtC[:, :],
                        in1=st["aliveB"][:, 128:256], accum_out=cmC[:, 0:1])
                    nc.vector._custom_dve(
                        MASKED_MAX, out=Wc[:, :], in0=Wc[:, :],
                        in1=st["aliveB"][:, 0:128], accum_out=rmC[:, 0:1])
                    nc.vector.tensor_tensor(out=st["v1"][:, :], in0=rmC[:, :],
                                            in1=st["ralC"][:, :], op=AL.mult)
                    nc.vector.tensor_tensor(out=st["v2"][:, :], in0=cmC[:, :],
                                            in1=st["calC"][:, :], op=AL.mult)

            def tail_b(st, r):
                rmG = st["rmC"] if r == 0 else st["v1"]
                cmG = st["cmC"] if r == 0 else st["v2"]
                bcast2([(cmG[:, 0:1], 0, 128)], [(rmG[:, 0:1], 0, 128)],
                       st["maxRow"], st["maxB"], wa=128, wb=128)

            def tail_c(st, r):
                Wc, WtC = st["Wc"], st["WtC"]
                rmG = st["rmC"] if r == 0 else st["v1"]
                cmG = st["cmC"] if r == 0 else st["v2"]
                nc.vector._custom_dve(
                    MUTUAL, out=trash[:, 0:128], in0=Wc[:, :],
                    in1=st["maxB"][:, 0:128], s0=rmG[:, 0:1],
                    accum_out=st["cntRC"][:, 0:1])
                nc.vector._custom_dve(
                    MUTUAL, out=trash[:, 0:128], in0=WtC[:, :],
                    in1=st["maxB"][:, 128:256], s0=cmG[:, 0:1],
                    accum_out=st["cntCC"][:, 0:1])
                kR, kC = st["cntRC"], st["cntCC"]
                nc.vector.tensor_scalar(out=kR[:, :], in0=kR[:, :],
                                        scalar1=0.5, scalar2=None,
                                        op0=AL.is_ge)
                nc.vector.tensor_scalar(out=kC[:, :], in0=kC[:, :],
                                        scalar1=0.5, scalar2=None,
                                        op0=AL.is_ge)
                t = st["v1"] if r == 0 else st["rmC"]
                nc.vector.tensor_tensor(out=t[:, :], in0=rmG[:, :],
                                        in1=kR[:, :], op=AL.mult)
                nc.vector.tensor_tensor(out=st["dRC"][:, :],
                                        in0=st["dRC"][:, :],
                                        in1=t[:, :], op=AL.add)
                nc.vector.tensor_scalar(out=kR[:, :], in0=kR[:, :],
                                        scalar1=-1.0, scalar2=1.0,
                                        op0=AL.mult, op1=AL.add)
                nc.vector.tensor_tensor(out=st["ralC"][:, :],
                                        in0=st["ralC"][:, :],
                                        in1=kR[:, :], op=AL.mult)
                nc.vector.tensor_scalar(out=kC[:, :], in0=kC[:, :],
                                        scalar1=-1.0, scalar2=1.0,
                                        op0=AL.mult, op1=AL.add)
                nc.vector.tensor_tensor(out=st["calC"][:, :],
                                        in0=st["calC"][:, :],
                                        in1=kC[:, :], op=AL.mult)

            def tail_d(st, r):
                if r + 1 < tail_rounds:
                    bcast2([(st["calC"][:, 0:1], 0, 128)],
                           [(st["ralC"][:, 0:1], 0, 128)],
                           st["alvRow"], st["aliveB"], wa=128, wb=128)

            def reload(st, m, blocks):
                for k in blocks:
                    nc.sync.dma_start(out=st["W"][k][:, :],
                                      in_=s_ap[m, 128 * k:128 * (k + 1), :])

            def output(st, m):
                # translate tail deaths (rid: orig row id per tail slot)
                pm = pool_psM.tile([128, 4], FP, name="ps", tag="ps")
                for k in range(4):
                    nc.vector.tensor_scalar(out=st["t1"][:, k:k + 1],
                                            in0=st["rid"][:, :],
                                            scalar1=float(-128 * k),
                                            scalar2=None, op0=AL.add)
                oh4 = [st["GrT"][0], st["GrT"][1], st["GcT"][0],
                       st["GcT"][1]]
                for k in range(4):
                    nc.vector.tensor_scalar(out=oh4[k][:, :],
                                            in0=iotaF128,
                                            scalar1=st["t1"][:, k:k + 1],
                                            scalar2=None, op0=AL.is_equal)
                for k in range(4):
                    nc.tensor.matmul(pm[:, k:k + 1], oh4[k][:, :],
                                     st["dRC"][:, 0:1],
                                     start=True, stop=True)
                nc.vector.tensor_tensor(out=st["deathR"][:, :],
                                        in0=st["deathR"][:, :],
                                        in1=pm[:, :], op=AL.add)
                # translate L1 deaths (rid1p: orig row id per L1 slot)
                pm2 = pool_psM.tile([128, 4], FP, name="ps", tag="ps")
                for b in range(2):
                    for k in range(4):
                        nc.vector.tensor_scalar(out=st["t2"][:, k:k + 1],
                                                in0=st["rid1p"][:, b:b + 1],
                                                scalar1=float(-128 * k),
                                                scalar2=None, op0=AL.add)
                    for k in range(4):
                        nc.vector.tensor_scalar(out=oh4[k][:, :],
                                                in0=iotaF128,
                                                scalar1=st["t2"][:, k:k + 1],
                                                scalar2=None, op0=AL.is_equal)
                    for k in range(4):
                        nc.tensor.matmul(pm2[:, k:k + 1], oh4[k][:, :],
                                         st["dR1"][:, b:b + 1],
                                         start=(b == 0), stop=(b == 1))
                nc.vector.tensor_tensor(out=st["deathR"][:, :],
                                        in0=st["deathR"][:, :],
                                        in1=pm2[:, :], op=AL.add)
                # one-hot by value against the reloaded original matrix
                for k in range(4):
                    ot = pool_out.tile([128, 512], FP, name=f"ot{k % 2}",
                                       tag=f"ot{k % 2}")
                    nc.vector.tensor_scalar(out=ot[:, :], in0=st["W"][k][:, :],
                                            scalar1=st["deathR"][:, k:k + 1],
                                            scalar2=None, op0=AL.is_equal)
                    nc.sync.dma_start(out=out_ap[m, 128 * k:128 * (k + 1), :],
                                      in_=ot[:, :])

            # ============ pipelined emission: 2 cohorts of csize ============
            csize = group // 2
            A = list(range(csize))
            Bc = list(range(csize, 2 * csize))

            def front_thunks(slots, mats):
                th = []
                for r in range(2):
                    for i, s in enumerate(slots):
                        th.append(lambda s=s, r=r: full_a(states[s], r))
                    for s in slots:
                        th.append(lambda s=s, r=r: full_b(states[s], r))
                    for s in slots:
                        th.append(lambda s=s, r=r: full_c(states[s], r))
                    for s in slots:
                        th.append(lambda s=s, r=r: full_d(states[s], r))
                for s in slots:
                    th.append(lambda s=s: compact1(states[s]))
                for s, m in zip(slots, mats):
                    th.append(lambda s=s, m=m: reload(states[s], m, [1, 2, 3]))
                for s in slots:
                    th.append(lambda s=s: l1_a(states[s]))
                for s in slots:
                    th.append(lambda s=s: l1_b(states[s]))
                for s in slots:
                    th.append(lambda s=s: l1_c(states[s]))
                for s in slots:
                    th.append(lambda s=s: compact2(states[s]))
                for s, m in zip(slots, mats):
                    th.append(lambda s=s, m=m: reload(states[s], m, [0]))
                return th

            def back_thunks(slots, mats):
                th = []
                for r in range(tail_rounds):
                    for s in slots:
                        th.append(lambda s=s, r=r: tail_a(states[s], r))
                    for s in slots:
                        th.append(lambda s=s, r=r: tail_b(states[s], r))
                    for s in slots:
                        th.append(lambda s=s, r=r: tail_c(states[s], r))
                    for s in slots:
                        th.append(lambda s=s, r=r: tail_d(states[s], r))
                for s, m in zip(slots, mats):
                    th.append(lambda s=s, m=m: output(states[s], m))
                return th

            def emit_interleaved(tx, ty):
                nx, ny = len(tx), len(ty)
                ix = iy = 0
                while ix < nx or iy < ny:
                    if ix < nx and (iy >= ny or ix * ny <= iy * nx):
                        tx[ix]()
                        ix += 1
                    else:
                        ty[iy]()
                        iy += 1

            batches = [list(range(n_mat))[i:i + csize]
                       for i in range(0, n_mat, csize)]
            # prologue: load + front for batch 0 on cohort A
            for s, m in zip(A, batches[0]):
                load(states[s], m)
            for t in range(len(batches) + 1):
                X = A if t % 2 == 0 else Bc            # front cohort (batch t)
                Y = Bc if t % 2 == 0 else A            # back cohort (batch t-1)
                tx = front_thunks(X, batches[t]) if t < len(batches) else []
                ty = back_thunks(Y, batches[t - 1]) if t >= 1 else []
                emit_interleaved(tx, ty)
                # load batch t+1 into cohort Y (free after its back phase)
                if t + 1 < len(batches):
                    for s, m in zip(Y, batches[t + 1]):
                        load(states[s], m)
    return nc


# ----------------------------------------------------------------------------
# Host-side entry point: shard the 256-matrix batch over 8 NeuronCores
# (pure data parallelism, 32 matrices per core), run the SPMD kernel,
# reassemble, verify each matrix (structural + greedy-order check), complete
# the small residuals exactly, and recompute any flagged matrix exactly.
# ----------------------------------------------------------------------------
from concourse.bass_utils import run_bass_kernel_spmd

N_CORES = 8
B, N = 256, 512
MPC = B // N_CORES  # matrices per core


def _greedy_ref_one(w):
    """Exact numpy mirror of the jax reference for one [N,N] matrix."""
    w = w.copy()
    perm = np.zeros_like(w)
    n = w.shape[0]
    for _ in range(n):
        flat = np.argmax(w)
        r, c = flat // n, flat % n
        perm[r, c] = 1.0
        w[r, :] = 0.0
        w[:, c] = 0.0
    return perm


_CACHE = {}


def _get_graph():
    if "nc" not in _CACHE:
        nc = bacc.Bacc()
        s_ext = nc.declare_dram_parameter("s", [MPC, N, N], FP, isOutput=False)
        c_ext = nc.declare_dram_parameter("consts", [128, CONST_W], FP,
                                          isOutput=False)
        o_ext = nc.declare_dram_parameter("out", [MPC, N, N], FP,
                                          isOutput=True)
        build_nms_kernel(nc, o_ext, s_ext, c_ext, n_mat=MPC)
        nc.finalize()
        _CACHE["nc"] = nc
    return _CACHE["nc"]


def kernel(s: np.ndarray) -> np.ndarray:
    s = np.ascontiguousarray(np.asarray(s), dtype=np.float32)
    assert s.shape == (B, N, N)
    nc = _get_graph()
    consts = make_consts()
    shards = s.reshape(N_CORES, MPC, N, N)
    in_maps = [{"s": shards[i], "consts": consts} for i in range(N_CORES)]
    res = run_bass_kernel_spmd(nc, in_maps, core_ids=list(range(N_CORES)))
    out = np.concatenate([np.asarray(res.results[i]["out"])
                          for i in range(N_CORES)], axis=0)
    out = out.reshape(B, N, N).astype(np.float32)
    # ---- host verification + exact completion --------------------------
    rs = out.sum(axis=2)
    cs = out.sum(axis=1)
    # greedy-order verifier: every cell must be <= max(deathR_row, deathC_col)
    # wherever at least one side is matched (prefix property of greedy).
    dR = (out * s).sum(axis=2)
    dC = (out * s).sum(axis=1)
    bound = np.maximum(dR[:, :, None], dC[:, None, :])
    matched_any = (dR[:, :, None] > 0) | (dC[:, None, :] > 0)
    viol = ((s > bound) & matched_any).any(axis=(1, 2))
    for b in range(B):
        bad = (viol[b] or (rs[b] > 1.0).any() or (cs[b] > 1.0).any())
        if not bad:
            ur = np.where(rs[b] == 0.0)[0]
            uc = np.where(cs[b] == 0.0)[0]
            bad = len(ur) != len(uc)
        if bad:
            out[b] = _greedy_ref_one(s[b])
            continue
        if len(ur) == 0:
            continue
        # continue greedy on the residual submatrix (exact: device matches
        # form a greedy prefix, verified above)
        sub = s[b][np.ix_(ur, uc)].copy()
        n = len(ur)
        for _ in range(n):
            flat = np.argmax(sub)
            r, c = flat // n, flat % n
            out[b, ur[r], uc[c]] = 1.0
            sub[r, :] = 0.0
            sub[:, c] = 0.0
    return out


# revision 48
# speedup vs baseline: 1.8605x; 1.8605x over previous
"""Greedy bipartite matching (NMS-style) Bass kernel for TRN2 — v2.

Algorithm: iterated locally-dominant (mutual-max) matching == sequential
greedy matching.  v2 replaces the index-key mutual test with a CELL-based
test fused into two custom DVE ops:

  MASKED_MAX : W = W * aliveB (in place), accum_out = per-partition max
  MUTUAL     : count = sum_j [ W[p,j] == ownmax_p  &  W[p,j] == othermaxB[j] ]

A cell is matched iff it equals both its row max and its column max
(checked per cell, so cross-row/col duplicate values cannot alias).  The
matched VALUE per row is recorded (deathR); the output permutation row is
recovered at the end as one_hot = (s_orig == deathR), which is exact
unless the death value is duplicated inside its own row (rare; caught by
the host structural check and recomputed exactly).

Stages per matrix: 2 full rounds (512-wide, round 1 uses the cheap
tensor_scalar max-accum since nothing is masked yet), compact to the
2-block L1 layout (dense ids, 160-wide windows), 1 L1 round, compact to a
single [128,128] tile, 1 tail round (PSUM diag-matmul broadcasts, no
Pool chain), host completes the ~42-row residual exactly (greedy prefix
property, checked by an O(N^2) order verifier; any violation triggers an
exact recompute).  No column-id bookkeeping is needed anywhere; row-id
maps (rid1p / rid) translate compact-space death values back to original
rows via one-hot matmuls at output time.

Emission is a 3-stage software pipeline over 3 cohorts of 2 matrices
(full rounds | compaction+L1 | tails+output), merged into one stream by
estimated DVE cost so the in-order engine queues stay busy.
"""

import numpy as np
import concourse.bass as bass
import concourse.bacc as bacc
import concourse.mybir as mybir
from concourse.tile import TileContext
from concourse import library_config

FP = mybir.dt.float32
AL = mybir.AluOpType

# ---------------------------------------------------------------------------
# custom DVE ops (registered into concourse.dve_ops at import; this is the
# documented extension path for ant custom ops)
# ---------------------------------------------------------------------------
import concourse.dve_ops as _dvo
from concourse.dve_ops import DveOp as _DveOp
from concourse.dve_spec import (Spec as _Spec, Src0 as _S0, Src1 as _S1,
                                C0 as _C0, maxx as _maxx, eq as _eq,
                                lower as _lower, AluOp as _AluOp)
from concourse.dve_uop import DveOpSpec as _DveOpSpec


def _register_op(name, spec, subdim=False):
    if name in _dvo._SUB_OPCODE_FOR_NAME:
        for op in _dvo.OPS:
            if op.name == name:
                return op
    shas = {}
    for ver in ("v3",):
        uops = _lower(spec, ver=ver)
        shas[ver] = _DveOpSpec(name=name, opcode=1, uops=uops,
                               rd1_en=True).sha(ver)
    op = _DveOp(name, spec, subdim, shas)
    _dvo.OPS.append(op)
    _dvo._SUB_OPCODE_FOR_NAME[name] = (_dvo._CUSTOM_DVE_ROW_BASE
                                       + len(_dvo.OPS) - 1)
    _dvo.CUSTOM_DVE_SPECS[name] = spec
    return op


def _ref_masked_max(in0, in1, c0, c1, c2):
    b = (in0 * in1).astype(np.float32)
    return b, b.reshape(b.shape[0], -1).max(axis=-1, keepdims=True)


def _ref_mutual(in0, in1, c0, c1, c2):
    b = ((in0 == c0) & (in0 == in1)).astype(np.float32)
    return b, b.reshape(b.shape[0], -1).sum(axis=-1, keepdims=True)


MASKED_MAX = _register_op(
    "NMS_MASKED_MAX",
    _Spec(body=_S0 * _S1, accum=_maxx, reference=_ref_masked_max))

MUTUAL = _register_op(
    "NMS_MUTUAL",
    _Spec(body=_eq(_S0, _C0) & _eq(_S0, _S1), accum=_AluOp.ADD,
          reference=_ref_mutual))


# ---- const layout (free-dim offsets into the [128, CONST_W] consts tensor)
OFF_I128 = 0         # [128,128] identity
OFF_UT128 = 128      # [128,128] upper-tri (q<=p)
OFF_IOTAF128 = 256   # [128,128] value f
OFF_IOTAF160 = 384   # [128,160] value f
OFF_ONES = 544       # [128,128] ones
OFF_COLID = 672      # [128,4] 128k+p
CONST_W = 676


def make_consts() -> np.ndarray:
    c = np.zeros((128, CONST_W), dtype=np.float32)
    q = np.arange(128)
    c[:, OFF_I128:OFF_I128 + 128] = np.eye(128, dtype=np.float32)
    c[:, OFF_UT128:OFF_UT128 + 128] = (q[:, None] <= q[None, :]).astype(np.float32)
    c[:, OFF_IOTAF128:OFF_IOTAF128 + 128] = q[None, :]
    c[:, OFF_IOTAF160:OFF_IOTAF160 + 160] = np.arange(160)[None, :]
    c[:, OFF_ONES:OFF_ONES + 128] = 1.0
    for k in range(4):
        c[:, OFF_COLID + k] = 128 * k + q
    return c


def build_nms_kernel(nc: bass.Bass, out_ap, s_ap, consts_ap, n_mat: int,
                     tail_rounds: int = 1, group: int = 6):
    with TileContext(nc) as tc:
        with (
            tc.tile_pool(name="consts", bufs=1) as pool_c,
            tc.tile_pool(name="big", bufs=1) as pool_big,
            tc.tile_pool(name="sm", bufs=1) as pool_sm,
            tc.tile_pool(name="vec", bufs=1) as pool_vec,
            tc.tile_pool(name="outp", bufs=1) as pool_out,
            tc.tile_pool(name="ps", bufs=2, space="PSUM") as pool_ps,
            tc.tile_pool(name="psM", bufs=2, space="PSUM") as pool_psM,
            tc.tile_pool(name="psT", bufs=1, space="PSUM") as pool_psT,
            tc.tile_pool(name="psC", bufs=1, space="PSUM") as pool_psC,
            tc.tile_pool(name="psB", bufs=1, space="PSUM") as pool_psB,
        ):
            C = pool_c.tile([128, CONST_W], FP, name="consts", tag="consts")
            nc.sync.dma_start(out=C[:, :], in_=consts_ap[:, :])
            I128 = C[:, OFF_I128:OFF_I128 + 128]
            UT128 = C[:, OFF_UT128:OFF_UT128 + 128]
            iotaF128 = C[:, OFF_IOTAF128:OFF_IOTAF128 + 128]
            iotaF160 = C[:, OFF_IOTAF160:OFF_IOTAF160 + 160]
            ONES = C[:, OFF_ONES:OFF_ONES + 128]
            iotaColId = C[:, OFF_COLID:OFF_COLID + 4]

            nc.gpsimd.load_library(library_config.proxy)
            warm = pool_psT.tile([128, 128], FP, name="warm", tag="pst")
            nc.tensor.transpose(warm[:, :], I128, I128)

            def big(nm, s, w=2048):
                return pool_big.tile([128, w], FP, name=f"{nm}{s}",
                                     tag=f"{nm}{s}")

            def sm(nm, s, w=128):
                return pool_sm.tile([128, w], FP, name=f"{nm}{s}",
                                    tag=f"{nm}{s}")

            def vec(nm, s, w=4, p=128):
                return pool_vec.tile([p, w], FP, name=f"{nm}{s}",
                                     tag=f"{nm}{s}")

            trash = pool_big.tile([128, 512], FP, name="trashG", tag="trashG")
            rowstage = [pool_vec.tile([1, 1024], FP, name=f"rstg{i}",
                                      tag=f"rstg{i}") for i in range(group)]

            def make_state(s):
                st = {"sid": s}
                st["Wb"] = big("W_", s)          # 4 blocks of [128,512]
                st["Wtb"] = big("Wt_", s)
                st["W"] = [st["Wb"][:, 512 * k:512 * (k + 1)] for k in range(4)]
                st["Wt"] = [st["Wtb"][:, 512 * k:512 * (k + 1)] for k in range(4)]
                st["maxB"] = big("mxB_", s, w=1024)
                st["aliveB"] = big("alB_", s, w=1024)
                st["maxRow"] = rowstage[s]
                st["alvRow"] = rowstage[s]
                for nm in ("rowalive", "colalive", "rowmax", "colmax",
                           "rowmaxG", "colmaxG", "cntR", "cntC", "deathR",
                           "t1", "t2", "t3"):
                    st[nm] = vec(nm + "_", s)
                # compact1 / L1 state
                st["GrT"] = [sm(f"GrT{k}_", s) for k in range(4)]
                st["GcT"] = [sm(f"GcT{k}_", s) for k in range(4)]
                st["rid1p"] = vec("r1p_", s, 2)
                st["rowmax1"] = vec("rm1_", s, 2)
                st["colmax1"] = vec("cm1_", s, 2)
                st["ral1"] = vec("ra1_", s, 2)
                st["cal1"] = vec("ca1_", s, 2)
                st["cnt1R"] = vec("c1R_", s, 2)
                st["cnt1C"] = vec("c1C_", s, 2)
                st["dR1"] = vec("dR1_", s, 2)
                st["u1"] = vec("u1_", s, 2)
                st["u2"] = vec("u2_", s, 2)
                st["scanrow"] = vec("scn_", s, 12, p=1)
                st["scanrow2"] = vec("sc2_", s, 12, p=1)
                # compact2 / tail state (overlay compact1 scratch: GrT/GcT
                # [2] and [3] are dead after compact1's gathers)
                st["Wc"] = st["GrT"][2]
                st["WtC"] = st["GrT"][3]
                st["A2"] = [st["GcT"][2], st["GcT"][3]]
                st["rid"] = vec("rid_", s, 1)
                st["rmC"] = vec("rmC_", s, 1)
                st["cmC"] = vec("cmC_", s, 1)
                st["ralC"] = vec("ralC_", s, 1)
                st["calC"] = vec("calC_", s, 1)
                st["cntRC"] = vec("cRC_", s, 1)
                st["cntCC"] = vec("cCC_", s, 1)
                st["dRC"] = vec("dRC_", s, 1)
                st["v1"] = vec("v1_", s, 1)
                st["v2"] = vec("v2_", s, 1)
                return st

            states = [make_state(s) for s in range(group)]

            def bcast2(vecs_a, vecs_b, rowt, B, wa=512, wb=512):
                """vecs_a -> rowt[0:wa] -> B[:, 0:wa]; vecs_b -> [wa:wa+wb].
                Two independent half-chains so the first half's consumer can
                start while the second half broadcasts."""
                pra = pool_ps.tile([1, 512], FP, name="ps", tag="ps")
                for v, off, w in vecs_a:
                    nc.tensor.matmul(pra[0:1, off:off + w], v, I128[:, 0:w],
                                     start=True, stop=True)
                nc.scalar.copy(rowt[0:1, 0:wa], pra[0:1, 0:wa])
                nc.gpsimd.partition_broadcast(B[:, 0:wa], rowt[0:1, 0:wa])
                prb = pool_ps.tile([1, 512], FP, name="ps", tag="ps")
                for v, off, w in vecs_b:
                    nc.tensor.matmul(prb[0:1, off:off + w], v, I128[:, 0:w],
                                     start=True, stop=True)
                nc.scalar.copy(rowt[0:1, wa:wa + wb], prb[0:1, 0:wb])
                nc.gpsimd.partition_broadcast(B[:, wa:wa + wb],
                                              rowt[0:1, wa:wa + wb])

            # ================= stages =================
            def load_dma(st, m):
                for k in range(4):
                    nc.sync.dma_start(out=st["W"][k][:, :],
                                      in_=s_ap[m, 128 * k:128 * (k + 1), :])

            def load_tr(st, r):
                pt = pool_psT.tile([128, 512], FP, name="pst", tag="pst")
                for k in range(4):
                    nc.tensor.transpose(
                        pt[:, 128 * k:128 * (k + 1)],
                        st["W"][k][:, 128 * r:128 * (r + 1)], I128)
                nc.scalar.copy(st["Wt"][r][:, :], pt[:, :])

            def load(st, m):
                load_dma(st, m)
                for r in range(4):
                    load_tr(st, r)

            def full_a(st, r):
                W, Wt = st["W"], st["Wt"]
                rowmax, colmax = st["rowmax"], st["colmax"]
                if r == 0:
                    for k in range(4):
                        nc.vector.tensor_scalar(
                            out=trash[:, :], in0=Wt[k][:, :], scalar1=1.0,
                            scalar2=0.0, op0=AL.mult, op1=AL.max,
                            accum_out=colmax[:, k:k + 1])
                    for k in range(4):
                        nc.vector.tensor_scalar(
                            out=trash[:, :], in0=W[k][:, :], scalar1=1.0,
                            scalar2=0.0, op0=AL.mult, op1=AL.max,
                            accum_out=rowmax[:, k:k + 1])
                else:
                    for k in range(4):
                        nc.vector._custom_dve(
                            MASKED_MAX, out=W[k][:, :], in0=W[k][:, :],
                            in1=st["aliveB"][:, 0:512],
                            accum_out=rowmax[:, k:k + 1])
                    for k in range(4):
                        nc.vector._custom_dve(
                            MASKED_MAX, out=Wt[k][:, :], in0=Wt[k][:, :],
                            in1=st["aliveB"][:, 512:1024],
                            accum_out=colmax[:, k:k + 1])
                    nc.vector.tensor_tensor(out=st["colmaxG"][:, :],
                                            in0=colmax[:, :],
                                            in1=st["colalive"][:, :],
                                            op=AL.mult)
                    nc.vector.tensor_tensor(out=st["rowmaxG"][:, :],
                                            in0=rowmax[:, :],
                                            in1=st["rowalive"][:, :],
                                            op=AL.mult)

            def full_b(st, r):
                rmG = st["rowmax"] if r == 0 else st["rowmaxG"]
                cmG = st["colmax"] if r == 0 else st["colmaxG"]
                bcast2([(cmG[:, k:k + 1], 128 * k, 128) for k in range(4)],
                       [(rmG[:, k:k + 1], 128 * k, 128) for k in range(4)],
                       st["maxRow"], st["maxB"])

            def full_c(st, r):
                W, Wt = st["W"], st["Wt"]
                rmG = st["rowmax"] if r == 0 else st["rowmaxG"]
                cmG = st["colmax"] if r == 0 else st["colmaxG"]
                for k in range(4):
                    nc.vector._custom_dve(
                        MUTUAL, out=trash[:, :], in0=W[k][:, :],
                        in1=st["maxB"][:, 0:512], s0=rmG[:, k:k + 1],
                        accum_out=st["cntR"][:, k:k + 1])
                for k in range(4):
                    nc.vector._custom_dve(
                        MUTUAL, out=trash[:, :], in0=Wt[k][:, :],
                        in1=st["maxB"][:, 512:1024], s0=cmG[:, k:k + 1],
                        accum_out=st["cntC"][:, k:k + 1])
                t1, t2, t3 = st["t1"], st["t2"], st["t3"]
                nc.vector.tensor_scalar(out=t1[:, :], in0=st["cntR"][:, :],
                                        scalar1=0.5, scalar2=None,
                                        op0=AL.is_ge)
                nc.vector.tensor_scalar(out=t2[:, :], in0=st["cntC"][:, :],
                                        scalar1=0.5, scalar2=None,
                                        op0=AL.is_ge)
                if r == 0:
                    nc.vector.tensor_tensor(out=st["deathR"][:, :],
                                            in0=st["rowmax"][:, :],
                                            in1=t1[:, :], op=AL.mult)
                    nc.vector.tensor_scalar(out=st["rowalive"][:, :],
                                            in0=t1[:, :], scalar1=-1.0,
                                            scalar2=1.0, op0=AL.mult,
                                            op1=AL.add)
                    nc.vector.tensor_scalar(out=st["colalive"][:, :],
                                            in0=t2[:, :], scalar1=-1.0,
                                            scalar2=1.0, op0=AL.mult,
                                            op1=AL.add)
                else:
                    nc.vector.tensor_tensor(out=t3[:, :], in0=st["rowmaxG"][:, :],
                                            in1=t1[:, :], op=AL.mult)
                    nc.vector.tensor_tensor(out=st["deathR"][:, :],
                                            in0=st["deathR"][:, :],
                                            in1=t3[:, :], op=AL.add)
                    nc.vector.tensor_scalar(out=t1[:, :], in0=t1[:, :],
                                            scalar1=-1.0, scalar2=1.0,
                                            op0=AL.mult, op1=AL.add)
                    nc.vector.tensor_tensor(out=st["rowalive"][:, :],
                                            in0=st["rowalive"][:, :],
                                            in1=t1[:, :], op=AL.mult)
                    nc.vector.tensor_scalar(out=t2[:, :], in0=t2[:, :],
                                            scalar1=-1.0, scalar2=1.0,
                                            op0=AL.mult, op1=AL.add)
                    nc.vector.tensor_tensor(out=st["colalive"][:, :],
                                            in0=st["colalive"][:, :],
                                            in1=t2[:, :], op=AL.mult)

            def full_d(st, r):
                if r == 0:
                    bcast2([(st["colalive"][:, k:k + 1], 128 * k, 128)
                            for k in range(4)],
                           [(st["rowalive"][:, k:k + 1], 128 * k, 128)
                            for k in range(4)],
                           st["alvRow"], st["aliveB"])

            def block_offsets(alive4, tot, w=4):
                ptot = pool_psM.tile([1, w], FP, name="ps", tag="ps")
                nc.tensor.matmul(ptot[0:1, :], ONES[:, 0:1], alive4[:, :],
                                 start=True, stop=True)
                nc.vector.tensor_copy(tot[0:1, 0:w], ptot[0:1, :])
                nc.vector.tensor_tensor_scan(
                    out=tot[0:1, 4:4 + w], data0=tot[0:1, 0:w],
                    data1=tot[0:1, 0:w],
                    initial=0.0, op0=AL.add, op1=AL.bypass)
                nc.vector.tensor_tensor(out=tot[0:1, 8:8 + w],
                                        in0=tot[0:1, 4:4 + w],
                                        in1=tot[0:1, 0:w], op=AL.subtract)
                pb = pool_psM.tile([128, w], FP, name="ps", tag="ps")
                nc.tensor.matmul(pb[:, :], ONES[0:1, 0:128],
                                 tot[0:1, 8:8 + w], start=True, stop=True)
                return pb

            # ---------- compact 512-space -> L1 2-block space ----------
            def compact1a(st):
                ppre = pool_psM.tile([128, 4], FP, name="ps", tag="ps")
                nc.tensor.matmul(ppre[:, :], UT128, st["rowalive"][:, :],
                                 start=True, stop=True)
                posR = st["t1"]
                nc.scalar.copy(posR[:, :], ppre[:, :])
                ppre2 = pool_psM.tile([128, 4], FP, name="ps", tag="ps")
                nc.tensor.matmul(ppre2[:, :], UT128, st["colalive"][:, :],
                                 start=True, stop=True)
                posC = st["t3"]
                nc.scalar.copy(posC[:, :], ppre2[:, :])
                offRB = block_offsets(st["rowalive"], st["scanrow"])
                offCB = block_offsets(st["colalive"], st["scanrow2"])
                nc.vector.tensor_tensor(out=posR[:, :], in0=posR[:, :],
                                        in1=offRB[:, :], op=AL.add)
                nc.vector.tensor_scalar(out=posR[:, :], in0=posR[:, :],
                                        scalar1=-1.0, scalar2=None,
                                        op0=AL.add)
                nc.vector.tensor_tensor(out=posC[:, :], in0=posC[:, :],
                                        in1=offCB[:, :], op=AL.add)
                nc.vector.tensor_scalar(out=posC[:, :], in0=posC[:, :],
                                        scalar1=-1.0, scalar2=None,
                                        op0=AL.add)
                posRm = st["t2"]
                nc.vector.tensor_scalar(out=posRm[:, :], in0=posR[:, :],
                                        scalar1=-128.0, scalar2=None,
                                        op0=AL.add)
                W1, Wt1 = st["W"][0], st["Wt"][0]
                GcTf = [st["W"][2][:, 0:160], st["W"][2][:, 160:320],
                        st["W"][2][:, 320:480], st["W"][3][:, 0:160]]
                Asb = [st["W"][1][:, 0:160], st["W"][1][:, 160:320],
                       st["W"][1][:, 320:480], st["W"][3][:, 160:320]]
                for cb in range(4):
                    nc.vector.tensor_scalar(out=GcTf[cb], in0=iotaF160,
                                            scalar1=posC[:, cb:cb + 1],
                                            scalar2=st["colalive"][:, cb:cb + 1],
                                            op0=AL.is_equal, op1=AL.mult)
                for k in range(4):
                    nc.vector.tensor_scalar(out=st["GrT"][k][:, :],
                                            in0=iotaF128,
                                            scalar1=posR[:, k:k + 1],
                                            scalar2=st["rowalive"][:, k:k + 1],
                                            op0=AL.is_equal, op1=AL.mult)
                    nc.vector.tensor_scalar(out=st["GcT"][k][:, :],
                                            in0=iotaF128,
                                            scalar1=posRm[:, k:k + 1],
                                            scalar2=st["rowalive"][:, k:k + 1],
                                            op0=AL.is_equal, op1=AL.mult)
                st["_GcTf"] = GcTf
                st["_Asb"] = Asb

            def compact1b(st):
                Asb = st["_Asb"]
                GcTf = st["_GcTf"]
                for k in range(4):
                    pA = pool_psC.tile([128, 160], FP, name="psA", tag="psA")
                    for cb in range(4):
                        nc.tensor.matmul(pA[:, :],
                                         st["Wt"][cb][:, 128 * k:128 * (k + 1)],
                                         GcTf[cb], start=(cb == 0),
                                         stop=(cb == 3))
                    nc.scalar.copy(Asb[k], pA[:, :])

            def compact1c(st):
                W1, Wt1 = st["W"][0], st["Wt"][0]
                Asb = st["_Asb"]
                for b in range(2):
                    pB = pool_psC.tile([128, 160], FP, name="psB", tag="psA")
                    for k in range(4):
                        G = st["GrT"][k] if b == 0 else st["GcT"][k]
                        nc.tensor.matmul(pB[:, :], G[:, :], Asb[k],
                                         start=(k == 0), stop=(k == 3))
                    nc.scalar.copy(W1[:, 256 * b:256 * b + 160], pB[:, :])
                # zero the stale pad regions so the Wt1 transposes below see
                # zeros (phantom slots then self-kill through the 0==0
                # channel at the L1 round and never reach compact2)
                nc.vector.memset(W1[:, 160:256], 0.0)
                nc.vector.memset(W1[:, 416:512], 0.0)
                ptW = pool_psT.tile([128, 512], FP, name="pst", tag="pst")
                for bp in range(2):
                    for seg in range(2):
                        nc.tensor.transpose(
                            ptW[:, 256 * bp + 128 * seg:256 * bp + 128 * seg + 128],
                            W1[:, 256 * seg + 128 * bp:256 * seg + 128 * bp + 128],
                            I128)
                nc.scalar.copy(Wt1[:, :], ptW[:, :])
                for b in range(2):
                    pr_ = pool_psM.tile([128, 1], FP, name="ps", tag="ps")
                    for k in range(4):
                        G = st["GrT"][k] if b == 0 else st["GcT"][k]
                        nc.tensor.matmul(pr_[:, :], G[:, :],
                                         iotaColId[:, k:k + 1],
                                         start=(k == 0), stop=(k == 3))
                    nc.scalar.copy(st["rid1p"][:, b:b + 1], pr_[:, :])
                nc.vector.memset(st["dR1"][:, :], 0.0)

            # ---------- the single L1 round (everything alive) ----------
            def l1_a(st):
                W1, Wt1 = st["W"][0], st["Wt"][0]
                rm1, cm1 = st["rowmax1"], st["colmax1"]
                for b in range(2):
                    nc.vector.tensor_scalar(
                        out=trash[:, 0:160], in0=Wt1[:, 256 * b:256 * b + 160],
                        scalar1=1.0, scalar2=0.0, op0=AL.mult, op1=AL.max,
                        accum_out=cm1[:, b:b + 1])
                for b in range(2):
                    nc.vector.tensor_scalar(
                        out=trash[:, 0:160], in0=W1[:, 256 * b:256 * b + 160],
                        scalar1=1.0, scalar2=0.0, op0=AL.mult, op1=AL.max,
                        accum_out=rm1[:, b:b + 1])

            def l1_b(st):
                rm1, cm1 = st["rowmax1"], st["colmax1"]
                bcast2([(cm1[:, 0:1], 0, 128), (cm1[:, 1:2], 128, 32)],
                       [(rm1[:, 0:1], 0, 128), (rm1[:, 1:2], 128, 32)],
                       st["maxRow"], st["maxB"], wa=160, wb=160)

            def l1_c(st):
                W1, Wt1 = st["W"][0], st["Wt"][0]
                rm1, cm1 = st["rowmax1"], st["colmax1"]
                for b in range(2):
                    nc.vector._custom_dve(
                        MUTUAL, out=trash[:, 0:160],
                        in0=W1[:, 256 * b:256 * b + 160],
                        in1=st["maxB"][:, 0:160], s0=rm1[:, b:b + 1],
                        accum_out=st["cnt1R"][:, b:b + 1])
                for b in range(2):
                    nc.vector._custom_dve(
                        MUTUAL, out=trash[:, 0:160],
                        in0=Wt1[:, 256 * b:256 * b + 160],
                        in1=st["maxB"][:, 160:320], s0=cm1[:, b:b + 1],
                        accum_out=st["cnt1C"][:, b:b + 1])
                u1, u2 = st["u1"], st["u2"]
                nc.vector.tensor_scalar(out=u1[:, :], in0=st["cnt1R"][:, :],
                                        scalar1=0.5, scalar2=None,
                                        op0=AL.is_ge)
                nc.vector.tensor_scalar(out=u2[:, :], in0=st["cnt1C"][:, :],
                                        scalar1=0.5, scalar2=None,
                                        op0=AL.is_ge)
                nc.vector.tensor_tensor(out=st["dR1"][:, :], in0=rm1[:, :],
                                        in1=u1[:, :], op=AL.mult)
                nc.vector.tensor_scalar(out=st["ral1"][:, :], in0=u1[:, :],
                                        scalar1=-1.0, scalar2=1.0,
                                        op0=AL.mult, op1=AL.add)
                nc.vector.tensor_scalar(out=st["cal1"][:, :], in0=u2[:, :],
                                        scalar1=-1.0, scalar2=1.0,
                                        op0=AL.mult, op1=AL.add)

            # ---------- compact L1 -> tail [128,128] space ----------
            def compact2a(st):
                W1, Wt1 = st["W"][0], st["Wt"][0]
                pp1 = pool_psM.tile([128, 2], FP, name="ps", tag="ps")
                nc.tensor.matmul(pp1[:, :], UT128, st["ral1"][:, :],
                                 start=True, stop=True)
                posR = st["u1"]
                nc.scalar.copy(posR[:, 0:2], pp1[:, :])
                pp2 = pool_psM.tile([128, 2], FP, name="ps", tag="ps")
                nc.tensor.matmul(pp2[:, :], UT128, st["cal1"][:, :],
                                 start=True, stop=True)
                posC = st["u2"]
                nc.scalar.copy(posC[:, 0:2], pp2[:, :])
                offRB = block_offsets(st["ral1"], st["scanrow"], w=2)
                offCB = block_offsets(st["cal1"], st["scanrow2"], w=2)
                nc.vector.tensor_tensor(out=posR[:, 0:2], in0=posR[:, 0:2],
                                        in1=offRB[:, :], op=AL.add)
                nc.vector.tensor_scalar(out=posR[:, 0:2], in0=posR[:, 0:2],
                                        scalar1=-1.0, scalar2=None,
                                        op0=AL.add)
                nc.vector.tensor_tensor(out=posC[:, 0:2], in0=posC[:, 0:2],
                                        in1=offCB[:, :], op=AL.add)
                nc.vector.tensor_scalar(out=posC[:, 0:2], in0=posC[:, 0:2],
                                        scalar1=-1.0, scalar2=None,
                                        op0=AL.add)
                for b in range(2):
                    nc.vector.tensor_scalar(out=st["GrT"][b][:, :],
                                            in0=iotaF128,
                                            scalar1=posR[:, b:b + 1],
                                            scalar2=st["ral1"][:, b:b + 1],
                                            op0=AL.is_equal, op1=AL.mult)
                    nc.vector.tensor_scalar(out=st["GcT"][b][:, :],
                                            in0=iotaF128,
                                            scalar1=posC[:, b:b + 1],
                                            scalar2=st["cal1"][:, b:b + 1],
                                            op0=AL.is_equal, op1=AL.mult)

            def compact2b(st):
                W1, Wt1 = st["W"][0], st["Wt"][0]
                for b in range(2):
                    pA = pool_psT.tile([128, 128], FP, name="pst", tag="pst")
                    for cb in range(2):
                        nc.tensor.matmul(
                            pA[:, :],
                            Wt1[:, 256 * cb + 128 * b:256 * cb + 128 * b + 128],
                            st["GcT"][cb][:, :],
                            start=(cb == 0), stop=(cb == 1))
                    nc.scalar.copy(st["A2"][b][:, :], pA[:, :])
                pW = pool_psM.tile([128, 128], FP, name="ps", tag="ps")
                for b in range(2):
                    nc.tensor.matmul(pW[:, :], st["GrT"][b][:, :],
                                     st["A2"][b][:, :],
                                     start=(b == 0), stop=(b == 1))
                nc.scalar.copy(st["Wc"][:, :], pW[:, :])
                ptc = pool_psM.tile([128, 128], FP, name="ps", tag="ps")
                nc.tensor.transpose(ptc[:, :], st["Wc"][:, :], I128)
                nc.scalar.copy(st["WtC"][:, :], ptc[:, :])
                prid = pool_psM.tile([128, 1], FP, name="ps", tag="ps")
                for b in range(2):
                    nc.tensor.matmul(prid[:, :], st["GrT"][b][:, :],
                                     st["rid1p"][:, b:b + 1],
                                     start=(b == 0), stop=(b == 1))
                nc.scalar.copy(st["rid"][:, :], prid[:, :])
                nc.vector.memset(st["dRC"][:, :], 0.0)
                nc.vector.memset(st["ralC"][:, :], 1.0)
                nc.vector.memset(st["calC"][:, :], 1.0)

            def tail_a(st, r):
                Wc, WtC = st["Wc"], st["WtC"]
                rmC, cmC = st["rmC"], st["cmC"]
                if r == 0:
                    nc.vector.tensor_scalar(
                        out=trash[:, 0:128], in0=WtC[:, :], scalar1=1.0,
                        scalar2=0.0, op0=AL.mult, op1=AL.max,
                        accum_out=cmC[:, 0:1])
                    nc.vector.tensor_scalar(
                        out=trash[:, 0:128], in0=Wc[:, :], scalar1=1.0,
                        scalar2=0.0, op0=AL.mult, op1=AL.max,
                        accum_out=rmC[:, 0:1])
                else:
                    pb = st["alvBC"]
                    nc.vector._custom_dve(
                        MASKED_MAX, out=WtC[:, :], in0=WtC[:, :],
                        in1=pb[:, 384:512], accum_out=cmC[:, 0:1])
                    nc.vector._custom_dve(
                        MASKED_MAX, out=Wc[:, :], in0=Wc[:, :],
                        in1=pb[:, 256:384], accum_out=rmC[:, 0:1])
                    nc.vector.tensor_tensor(out=st["v1"][:, :], in0=rmC[:, :],
                                            in1=st["ralC"][:, :], op=AL.mult)
                    nc.vector.tensor_tensor(out=st["v2"][:, :], in0=cmC[:, :],
                                            in1=st["calC"][:, :], op=AL.mult)

            def tail_bP(st, r):
                # build diag(cmG)/diag(rmG) on Pool, broadcast via ONES matmul
                rmG = st["rmC"] if r == 0 else st["v1"]
                cmG = st["cmC"] if r == 0 else st["v2"]
                dg = st["GcT"][0]
                nc.scalar.activation(dg[:, :], I128,
                                     mybir.ActivationFunctionType.Copy,
                                     scale=cmG[:, 0:1])
                dg2 = st["GcT"][1]
                nc.scalar.activation(dg2[:, :], I128,
                                     mybir.ActivationFunctionType.Copy,
                                     scale=rmG[:, 0:1])
                pb = pool_psB.tile([128, 512], FP, name=f"psb{st['tj']}",
                                   tag=f"psb{st['tj']}")
                nc.tensor.matmul(pb[:, 0:128], ONES, dg[:, :],
                                 start=True, stop=True)
                nc.tensor.matmul(pb[:, 128:256], ONES, dg2[:, :],
                                 start=True, stop=True)
                st["maxBC"] = pb

            def tail_c(st, r):
                Wc, WtC = st["Wc"], st["WtC"]
                rmG = st["rmC"] if r == 0 else st["v1"]
                cmG = st["cmC"] if r == 0 else st["v2"]
                pb = st["maxBC"]
                nc.vector._custom_dve(
                    MUTUAL, out=trash[:, 0:128], in0=Wc[:, :],
                    in1=pb[:, 0:128], s0=rmG[:, 0:1],
                    accum_out=st["cntRC"][:, 0:1])
                nc.vector._custom_dve(
                    MUTUAL, out=trash[:, 0:128], in0=WtC[:, :],
                    in1=pb[:, 128:256], s0=cmG[:, 0:1],
                    accum_out=st["cntCC"][:, 0:1])
                kR, kC = st["cntRC"], st["cntCC"]
                nc.vector.tensor_scalar(out=kR[:, :], in0=kR[:, :],
                                        scalar1=0.5, scalar2=None,
                                        op0=AL.is_ge)
                nc.vector.tensor_scalar(out=kC[:, :], in0=kC[:, :],
                                        scalar1=0.5, scalar2=None,
                                        op0=AL.is_ge)
                t = st["v1"] if r == 0 else st["rmC"]
                nc.vector.tensor_tensor(out=t[:, :], in0=rmG[:, :],
                                        in1=kR[:, :], op=AL.mult)
                nc.vector.tensor_tensor(out=st["dRC"][:, :],
                                        in0=st["dRC"][:, :],
                                        in1=t[:, :], op=AL.add)
                nc.vector.tensor_scalar(out=kR[:, :], in0=kR[:, :],
                                        scalar1=-1.0, scalar2=1.0,
                                        op0=AL.mult, op1=AL.add)
                nc.vector.tensor_tensor(out=st["ralC"][:, :],
                                        in0=st["ralC"][:, :],
                                        in1=kR[:, :], op=AL.mult)
                nc.vector.tensor_scalar(out=kC[:, :], in0=kC[:, :],
                                        scalar1=-1.0, scalar2=1.0,
                                        op0=AL.mult, op1=AL.add)
                nc.vector.tensor_tensor(out=st["calC"][:, :],
                                        in0=st["calC"][:, :],
                                        in1=kC[:, :], op=AL.mult)

            def tail_dP(st, r):
                if r + 1 >= tail_rounds:
                    return
                dg = st["GcT"][0]
                nc.scalar.activation(dg[:, :], I128,
                                     mybir.ActivationFunctionType.Copy,
                                     scale=st["calC"][:, 0:1])
                dg2 = st["GcT"][1]
                nc.scalar.activation(dg2[:, :], I128,
                                     mybir.ActivationFunctionType.Copy,
                                     scale=st["ralC"][:, 0:1])
                pb = pool_psB.tile([128, 512], FP, name=f"psb{st['tj']}",
                                   tag=f"psb{st['tj']}")
                nc.tensor.matmul(pb[:, 256:384], ONES, dg[:, :],
                                 start=True, stop=True)
                nc.tensor.matmul(pb[:, 384:512], ONES, dg2[:, :],
                                 start=True, stop=True)
                st["alvBC"] = pb

            def reload(st, m, blocks):
                for k in blocks:
                    nc.sync.dma_start(out=st["W"][k][:, :],
                                      in_=s_ap[m, 128 * k:128 * (k + 1), :])

            def output(st, m):
                # translate tail deaths (rid: orig row id per tail slot)
                pm = pool_psM.tile([128, 4], FP, name="ps", tag="ps")
                for k in range(4):
                    nc.vector.tensor_scalar(out=st["t1"][:, k:k + 1],
                                            in0=st["rid"][:, :],
                                            scalar1=float(-128 * k),
                                            scalar2=None, op0=AL.add)
                oh4 = [st["GrT"][0], st["GrT"][1], st["GcT"][0],
                       st["GcT"][1]]
                for k in range(4):
                    nc.vector.tensor_scalar(out=oh4[k][:, :],
                                            in0=iotaF128,
                                            scalar1=st["t1"][:, k:k + 1],
                                            scalar2=None, op0=AL.is_equal)
                for k in range(4):
                    nc.tensor.matmul(pm[:, k:k + 1], oh4[k][:, :],
                                     st["dRC"][:, 0:1],
                                     start=True, stop=True)
                nc.vector.tensor_tensor(out=st["deathR"][:, :],
                                        in0=st["deathR"][:, :],
                                        in1=pm[:, :], op=AL.add)
                # translate L1 deaths (rid1p: orig row id per L1 slot)
                pm2 = pool_psM.tile([128, 4], FP, name="ps", tag="ps")
                for b in range(2):
                    for k in range(4):
                        nc.vector.tensor_scalar(out=st["t2"][:, k:k + 1],
                                                in0=st["rid1p"][:, b:b + 1],
                                                scalar1=float(-128 * k),
                                                scalar2=None, op0=AL.add)
                    for k in range(4):
                        nc.vector.tensor_scalar(out=oh4[k][:, :],
                                                in0=iotaF128,
                                                scalar1=st["t2"][:, k:k + 1],
                                                scalar2=None, op0=AL.is_equal)
                    for k in range(4):
                        nc.tensor.matmul(pm2[:, k:k + 1], oh4[k][:, :],
                                         st["dR1"][:, b:b + 1],
                                         start=(b == 0), stop=(b == 1))
                nc.vector.tensor_tensor(out=st["deathR"][:, :],
                                        in0=st["deathR"][:, :],
                                        in1=pm2[:, :], op=AL.add)
                # one-hot by value against the reloaded original matrix,
                # guarded by deathR > 0 (s may contain exact 0.0 cells, and
                # unmatched rows carry deathR == 0)
                nc.vector.tensor_scalar(out=st["t3"][:, :],
                                        in0=st["deathR"][:, :],
                                        scalar1=0.0, scalar2=None,
                                        op0=AL.is_gt)
                for k in range(4):
                    ot = pool_out.tile([128, 512], FP, name=f"ot{k % 2}",
                                       tag=f"ot{k % 2}")
                    nc.vector.tensor_scalar(out=ot[:, :], in0=st["W"][k][:, :],
                                            scalar1=st["deathR"][:, k:k + 1],
                                            scalar2=st["t3"][:, k:k + 1],
                                            op0=AL.is_equal, op1=AL.mult)
                    nc.sync.dma_start(out=out_ap[m, 128 * k:128 * (k + 1), :],
                                      in_=ot[:, :])

            # ====== 3-stage pipelined emission: cohorts of csize slots ======
            csize = group // 3
            cohorts = [list(range(i * csize, (i + 1) * csize))
                       for i in range(3)]

            def s1_thunks(slots, mats):
                th = []
                for r in range(2):
                    for s in slots:
                        th.append(_lab(lambda s=s, r=r: full_a(states[s], r),
                                       'full_a0' if r == 0 else 'full_a1'))
                    for s in slots:
                        th.append(_lab(lambda s=s, r=r: full_b(states[s], r), 'full_b'))
                    for s in slots:
                        th.append(_lab(lambda s=s, r=r: full_c(states[s], r),
                                       'full_c0' if r == 0 else 'full_c1'))
                    for s in slots:
                        th.append(_lab(lambda s=s, r=r: full_d(states[s], r), 'full_d'))
                return th

            def s2_thunks(slots, mats):
                th = []
                for s in slots:
                    th.append(_lab(lambda s=s: compact1a(states[s]), 'compact1a'))
                for s in slots:
                    th.append(_lab(lambda s=s: compact1b(states[s]), 'compact1b'))
                for s in slots:
                    th.append(_lab(lambda s=s: compact1c(states[s]), 'compact1c'))
                for s, m in zip(slots, mats):
                    th.append(_lab(lambda s=s, m=m: reload(states[s], m, [1, 2, 3]), 'reload'))
                for s in slots:
                    th.append(_lab(lambda s=s: l1_a(states[s]), 'l1_a'))
                for s in slots:
                    th.append(_lab(lambda s=s: l1_b(states[s]), 'l1_b'))
                for s in slots:
                    th.append(_lab(lambda s=s: l1_c(states[s]), 'l1_c'))
                for s in slots:
                    th.append(_lab(lambda s=s: compact2a(states[s]), 'compact2a'))
                for s in slots:
                    th.append(_lab(lambda s=s: compact2b(states[s]), 'compact2b'))
                for s, m in zip(slots, mats):
                    th.append(_lab(lambda s=s, m=m: reload(states[s], m, [0]), 'reload'))
                return th

            def s3_thunks(slots, mats, next_mats=None):
                th = []
                sts = [states[s] for s in slots]
                for j, st in enumerate(sts):
                    st["tj"] = j
                for r in range(tail_rounds):
                    for st in sts:
                        th.append(_lab(lambda st=st, r=r: tail_a(st, r), 'tail_a'))
                    for st in sts:
                        th.append(_lab(lambda st=st, r=r: tail_bP(st, r), 'tail_b'))
                    for st in sts:
                        th.append(_lab(lambda st=st, r=r: tail_c(st, r), 'tail_c'))
                    for st in sts:
                        th.append(_lab(lambda st=st, r=r: tail_dP(st, r), 'tail_d'))
                for s, m in zip(slots, mats):
                    th.append(_lab(lambda s=s, m=m: output(states[s], m), 'output'))
                if next_mats is not None:
                    for s, m in zip(slots, next_mats):
                        th.append(_lab(lambda s=s, m=m: load(states[s], m), 'load'))
                return th

            DVEW = {"full_a0": 2.8, "full_a1": 5.2, "full_b": 0.01, "full_c0": 5.6, "full_c1": 6.2,
                    "full_d": 0.01, "compact1a": 0.7, "compact1b": 0.05,
                    "compact1c": 0.05, "reload": 0.01,
                    "l1_a": 0.25, "l1_b": 0.01, "l1_c": 0.5,
                    "compact2a": 0.4, "compact2b": 0.1, "tail_a": 0.25,
                    "tail_b": 0.01, "tail_c": 0.3, "tail_d": 0.01,
                    "output": 1.1, "load": 1.6}

            def emit_merged(streams):
                import kernel as _K
                streams = [s for s in streams if s]
                ws = [[DVEW.get(t.__qualname__, 0.1) for t in s]
                      for s in streams]
                tots = [max(sum(w), 1e-9) for w in ws]
                cums = [0.0] * len(streams)
                idx = [0] * len(streams)
                while True:
                    best = -1
                    bestfrac = None
                    for i, s in enumerate(streams):
                        if idx[i] < len(s):
                            frac = cums[i] / tots[i]
                            if bestfrac is None or frac < bestfrac:
                                bestfrac = frac
                                best = i
                    if best < 0:
                        break
                    t = streams[best][idx[best]]
                    i0 = nc.next_id()
                    t()
                    _K.PHASELOG.append((i0, nc.next_id(), t.__qualname__))
                    cums[best] += ws[best][idx[best]]
                    idx[best] += 1

            batches = [list(range(n_mat))[i:i + csize]
                       for i in range(0, n_mat, csize)]
            nb = len(batches)
            for s, m in zip(cohorts[0], batches[0]):
                load(states[s], m)
            for t in range(nb + 2):
                streams = []
                if t < nb:
                    streams.append(s1_thunks(cohorts[t % 3], batches[t]))
                if 1 <= t < nb + 1:
                    streams.append(s2_thunks(cohorts[(t - 1) % 3],
                                             batches[t - 1]))
                if 2 <= t:
                    nxt = batches[t + 1] if t + 1 < nb else None
                    streams.append(s3_thunks(cohorts[(t - 2) % 3],
                                             batches[t - 2], nxt))
                emit_merged(streams)
    return nc


# ----------------------------------------------------------------------------
# Host-side entry point: shard the 256-matrix batch over 8 NeuronCores
# (pure data parallelism, 32 matrices per core), run the SPMD kernel,
# reassemble, verify each matrix (structural + greedy-order check), complete
# the small residuals exactly, and recompute any flagged matrix exactly.
# ----------------------------------------------------------------------------
from concourse.bass_utils import run_bass_kernel_spmd

N_CORES = 8
B, N = 256, 512
MPC = B // N_CORES  # matrices per core


def _greedy_ref_one(w):
    """Exact numpy mirror of the jax reference for one [N,N] matrix."""
    w = w.copy()
    perm = np.zeros_like(w)
    n = w.shape[0]
    for _ in range(n):
        flat = np.argmax(w)
        r, c = flat // n, flat % n
        perm[r, c] = 1.0
        w[r, :] = 0.0
        w[:, c] = 0.0
    return perm


_CACHE = {}


def _get_graph():
    if "nc" not in _CACHE:
        nc = bacc.Bacc()
        s_ext = nc.declare_dram_parameter("s", [MPC, N, N], FP, isOutput=False)
        c_ext = nc.declare_dram_parameter("consts", [128, CONST_W], FP,
                                          isOutput=False)
        o_ext = nc.declare_dram_parameter("out", [MPC, N, N], FP,
                                          isOutput=True)
        build_nms_kernel(nc, o_ext, s_ext, c_ext, n_mat=MPC)
        nc.finalize()
        _CACHE["nc"] = nc
    return _CACHE["nc"]


def kernel(s: np.ndarray) -> np.ndarray:
    s = np.ascontiguousarray(np.asarray(s), dtype=np.float32)
    assert s.shape == (B, N, N)
    nc = _get_graph()
    consts = make_consts()
    shards = s.reshape(N_CORES, MPC, N, N)
    in_maps = [{"s": shards[i], "consts": consts} for i in range(N_CORES)]
    res = run_bass_kernel_spmd(nc, in_maps, core_ids=list(range(N_CORES)))
    out = np.concatenate([np.asarray(res.results[i]["out"])
                          for i in range(N_CORES)], axis=0)
    out = out.reshape(B, N, N).astype(np.float32)
    # ---- host verification + exact completion --------------------------
    rs = out.sum(axis=2)
    cs = out.sum(axis=1)
    # greedy-order verifier: every cell must be <= max(deathR_row, deathC_col)
    # wherever at least one side is matched (prefix property of greedy).
    dR = (out * s).sum(axis=2)
    dC = (out * s).sum(axis=1)
    bound = np.maximum(dR[:, :, None], dC[:, None, :])
    matched_any = (rs[:, :, None] > 0) | (cs[:, None, :] > 0)
    viol = ((s > bound) & matched_any).any(axis=(1, 2))
    for b in range(B):
        bad = (viol[b] or (rs[b] > 1.0).any() or (cs[b] > 1.0).any())
        if not bad:
            ur = np.where(rs[b] == 0.0)[0]
            uc = np.where(cs[b] == 0.0)[0]
            bad = len(ur) != len(uc)
        if bad:
            out[b] = _greedy_ref_one(s[b])
            continue
        if len(ur) == 0:
            continue
        # continue greedy on the residual submatrix (exact: device matches
        # form a greedy prefix, verified above)
        sub = s[b][np.ix_(ur, uc)].copy()
        n = len(ur)
        for _ in range(n):
            flat = np.argmax(sub)
            r, c = flat // n, flat % n
            out[b, ur[r], uc[c]] = 1.0
            sub[r, :] = 0.0
            sub[:, c] = 0.0
    return out
